# revision 1
# baseline (speedup 1.0000x reference)
"""Trainium2 Bass kernel for nn_GatedLinear (gated LoRA-MoE linear layer).

Math (see reference):
  base_out = x @ base_w.T + base_b
  logits   = x @ router_w.T ; top-2 softmax -> dense per-expert gate
  h        = x @ lora_A.T   ; rank_w = repeat(gate*scalings, 16)
  out      = base_out + (h * rank_w) @ lora_B.T

Sharding: pure data-parallel over batch*seq across 8 cores (1024 tokens
per core); all weights replicated. No collectives.

Device-side strategy (v5):
  * The host ships three copies of x.T: exact fp32 (router only --
    top-2 selection must match the fp32 reference bit-for-bit), bf16
    (h matmul), and fp8e4m3 (base matmul). fp32 chunks go through a
    small ring on two HWDGE queues; bf16/fp8 chunks DMA straight into
    resident tiles on the gpsimd queue. Sub-tile deps let every
    consumer start as soon as its ko-chunk lands.
  * Base matmul runs fp8e4m3 with perf_mode=DoubleRow: weights are
    host-scaled x64 into e4m3's range and packed [k2, 2, f] so each
    matmul contracts 256 deep -- half the instructions of bf16, 216ns
    per 256x128x512 step. The x64 scale is folded into the e8 gate
    expansion (so the lora step accumulates at the same scale) and
    removed in the bias epilogue (acc/64 + b, one DVE op).
  * lora_B.T stays resident bf16 and closes each PSUM accumulation
    group as a 33rd step with the gated rank activations; lora_A/h and
    the gating chain stay bf16/fp32, keeping total rel err ~1.1e-2
    (gate is 2e-2) with the fp8 noise confined to the base term.
  * Router matmuls (exact fp32) and h matmuls interleave across the
    ko-chunks so the PE stays busy while x streams in; gating runs
    token-major (PE transposes + DVE top-2 chain).
  * Weight DRAM layout is [ot, p, (k2 j f)] so each per-ot weight DMA
    is 128 contiguous runs (fast descriptor gen + full DMA bw).
  * DMA queues: fp32 x alternates sync/scalar; bf16+fp8 x, lora
    consts, and the weight stream on gpsimd; outputs on sync.

Output is produced transposed ([out_features, tokens] per core) and
de-transposed on the host.
"""

from contextlib import ExitStack

import numpy as np


def _ensure_path():
    try:
        import concourse.bass  # noqa: F401
    except ImportError:
        import sys

        for p in ("/opt/trn_rl_repo", "/root/.axon_site/_ro/trn_rl_repo"):
            if p not in sys.path:
                sys.path.insert(0, p)


N_CORES = 8
B, S, D, O = 4, 2048, 4096, 4096
T = B * S              # 8192 tokens total
T_PC = T // N_CORES    # 1024 tokens per core
E = 8                  # experts
RANK = 16
R = E * RANK           # 128 fused rank dim
P = 128
KO = D // P            # 32 k-subtiles of the contraction dim
KO_EXT = KO + 1        # +1 subtile holding lora_B.T
OTILES = O // P        # 32 output-feature tiles
TTILE = 512            # tokens per matmul moving operand
NT = T_PC // TTILE     # 2 token tiles per core
GT = 512               # gating token-tile size (512-row router streams
                       # hide LDWEIGHTS under the 4-term fp16 matmuls)
NGT = T_PC // GT       # 4 gating tiles
NGC = GT // P          # 128-chunks per gating tile

WT_BF16 = True         # bf16 stationary weights (mixed with f32r moving)
FP8_BASE = True        # fp8e4m3 DoubleRow for the base matmul (2x PE rate)
KO2 = KO // 2          # paired k-subtiles for DoubleRow (256-deep contraction)
W8_SCALE = 64.0        # base_w std is 1/64; scale into e4m3's sweet spot

_prog_cache = {}


def _build_program():
    """Build the single-core SPMD Bass program (same on all 8 cores)."""
    _ensure_path()
    import concourse.bass as bass
    import concourse.mybir as mybir
    import concourse.tile as tile
    from concourse import bacc

    f32 = mybir.dt.float32
    f32r = mybir.dt.float32r
    bf16 = mybir.dt.bfloat16
    f8 = mybir.dt.float8e4
    wdt = bf16 if WT_BF16 else f32r
    Alu = mybir.AluOpType
    Act = mybir.ActivationFunctionType
    DR = mybir.MatmulPerfMode.DoubleRow

    nc = bacc.Bacc(
        "TRN2",
        target_bir_lowering=False,
        debug=False,
        num_devices=N_CORES,
    )

    f16 = mybir.dt.float16
    # x ships as an fp16 hi/lo split: xh + rh == x to ~2^-23 relative,
    # so the 4-term fp16 router matches fp32 top-2 selection, at half
    # the HBM bytes of the old fp32 stream + bf16 copy.
    xh = nc.dram_tensor("xh", [D, T_PC], f16, kind="ExternalInput").ap()
    rh = nc.dram_tensor("rh", [D, T_PC], f16, kind="ExternalInput").ap()
    x8d = nc.dram_tensor("x8", [D, T_PC], f8, kind="ExternalInput").ap()
    xh_v = xh.rearrange("(ko p) t -> p ko t", p=P)
    rh_v = rh.rearrange("(ko p) t -> p ko t", p=P)
    x8_v = x8d.rearrange("(ko p) t -> p ko t", p=P)
    if FP8_BASE:
        # base weights only, x64-scaled fp8, DoubleRow pair layout
        wt = nc.dram_tensor(
            "wt", [OTILES * P, KO2 * 2 * P], f8, kind="ExternalInput"
        ).ap()
        wt_v = wt.rearrange("(ot p) (k j f) -> p ot k j f", p=P, j=2, f=P)
        lb = nc.dram_tensor("lb", [P, O], wdt, kind="ExternalInput").ap()
    else:
        wt = nc.dram_tensor(
            "wt", [OTILES * P, KO_EXT * P], wdt, kind="ExternalInput"
        ).ap()
        wt_v = wt.rearrange("(ot p) (ko f) -> p ot ko f", p=P, f=P)
        lb = None
    ar = nc.dram_tensor("ar", [P, KO * R], f16, kind="ExternalInput").ap()
    # router_w.T x64, fp16 hi/lo split (x64 keeps the lo part in fp16's
    # normal range; the scale is undone before the softmax)
    rwh = nc.dram_tensor("rwh", [P, KO * E], f16, kind="ExternalInput").ap()
    rwr = nc.dram_tensor("rwr", [P, KO * E], f16, kind="ExternalInput").ap()
    bb = nc.dram_tensor("bb", [O], f32, kind="ExternalInput").ap()
    e8 = nc.dram_tensor("e8", [E, P], f32, kind="ExternalInput").ap()
    idm = nc.dram_tensor("idm", [P, P], f32, kind="ExternalInput").ap()
    yt = nc.dram_tensor("yt", [O, T_PC], f32, kind="ExternalOutput").ap()

    ar_v = ar.rearrange("p (ko r) -> p ko r", r=R)        # [128, 32, 128]
    rwh_v = rwh.rearrange("p (ko e) -> p ko e", e=E)      # [128, 32, 8]
    rwr_v = rwr.rearrange("p (ko e) -> p ko e", e=E)
    bb_v = bb.rearrange("(ot p) -> p ot", p=P)            # [128, 32]
    yt_v = yt.rearrange("(ot p) t -> p ot t", p=P)        # [128, 32, 1024]

    with tile.TileContext(nc) as tc:
        with (
            tc.tile_pool(name="perm", bufs=1) as pp,
            tc.tile_pool(name="obuf", bufs=3) as ob,
            tc.tile_pool(name="wstream", bufs=3) as wpool,
        ):
            # router weights first on the scalar queue so its next instr
            # is the ko=1 x chunk (PE's second dependency); the other
            # consts ride gpsimd -- they aren't needed until gating
            rwhsb = pp.tile([P, KO, E], f16)
            nc.scalar.dma_start(rwhsb[:], rwh_v[:])
            rwrsb = pp.tile([P, KO, E], f16)
            nc.scalar.dma_start(rwrsb[:], rwr_v[:])
            bbsb = pp.tile([P, OTILES], f32)
            nc.gpsimd.dma_start(bbsb[:], bb_v[:])
            e8sb = pp.tile([E, P], f32)
            nc.gpsimd.dma_start(e8sb[:], e8[:])
            idsb = pp.tile([P, P], f32)
            nc.gpsimd.dma_start(idsb[:], idm[:])

            rgp = pp.tile([P, T_PC], bf16)   # per-rank gates [r, t]
            hwsb = pp.tile([P, T_PC], wdt)   # gated rank activations [r, t]

            # resident fp16 hi/lo copies of x (router + h)
            xhsb = pp.tile([P, KO, T_PC], f16)
            rhsb = pp.tile([P, KO, T_PC], f16)
            if FP8_BASE:
                # resident fp8 copy of x (for the base matmul)
                x8sb = pp.tile([P, KO, T_PC], f8)
                # lora_B.T resident bf16 (stationary of the lora matmul)
                lbsb = pp.tile([P, O], wdt)
                nc.gpsimd.dma_start(lbsb[:], lb[:])

            # lora_A.T (fp16 stationary), gpsimd queue
            arsb = pp.tile([P, KO, R], f16)
            nc.gpsimd.dma_start(arsb[:], ar_v[:])

            # prefetch the first weight tiles at the head of the gpsimd
            # queue (ahead of the x8 blocks) so the main loop never
            # starts LDWEIGHTS-starved
            pre_w = []
            if FP8_BASE:
                for ot in range(3):
                    wsb = wpool.tile([P, KO2, 2, P], f8, tag="w")
                    nc.scalar.dma_start(wsb[:], wt_v[:, ot, :, :, :])
                    pre_w.append(wsb)

            # ---- phase 1: stream x; router logits (fp16 4-term) ----
            phase1 = ExitStack()
            gp = phase1.enter_context(tc.tile_pool(name="gtmp", bufs=1))
            ps_l = phase1.enter_context(
                tc.tile_pool(name="ps_l", bufs=NGT, space="PSUM")
            )
            ps_b = phase1.enter_context(
                tc.tile_pool(name="ps_b", bufs=2, space="PSUM")
            )
            ps_h = phase1.enter_context(
                tc.tile_pool(name="ps_h", bufs=2, space="PSUM")
            )

            # interleave the 4 gating tiles' router matmuls and the h
            # matmuls per ko-chunk so the PE starts as soon as the first
            # x chunk lands; x alternates two DMA queues for bandwidth
            lgs_t = [
                ps_l.tile([E, GT], f32, tag="lg", name=f"lg{g}")
                for g in range(NGT)
            ]
            h_t = [
                ps_h.tile([P, TTILE], f32, tag="h", name=f"h{t}")
                for t in range(NT)
            ]
            for ko in range(KO):
                # x hi/lo/fp8 blocks across three DMA queues: 1-ko blocks
                # for the first chunks (PE primes sooner), 2-ko after
                if ko < 4 or ko % 2 == 0:
                    kb = slice(ko, ko + 1) if ko < 4 else slice(ko, ko + 2)
                    nc.sync.dma_start(xhsb[:, kb, :], xh_v[:, kb, :])
                    nc.gpsimd.dma_start(rhsb[:, kb, :], rh_v[:, kb, :])
                if FP8_BASE and ko % 4 == 0:
                    k4 = slice(ko, ko + 4)
                    nc.scalar.dma_start(x8sb[:, k4, :], x8_v[:, k4, :])
                for g in range(NGT):
                    gs = slice(g * GT, (g + 1) * GT)
                    # 3 terms: the dropped rwr*rh term is ~2^-23 of logit
                    # scale, five orders below the top-2 flip threshold
                    # (HW-verified: identical output with and without it)
                    for ti, (rw_t, x_t) in enumerate(
                        (
                            (rwhsb, xhsb),
                            (rwhsb, rhsb),
                            (rwrsb, xhsb),
                        )
                    ):
                        nc.tensor.matmul(
                            lgs_t[g][:],
                            lhsT=rw_t[:, ko, :],
                            rhs=x_t[:, ko, gs],
                            start=(ko == 0 and ti == 0),
                            stop=(ko == KO - 1 and ti == 2),
                        )
                for tt in range(NT):
                    ts = slice(tt * TTILE, (tt + 1) * TTILE)
                    nc.tensor.matmul(
                        h_t[tt][:],
                        lhsT=arsb[:, ko, :],
                        rhs=xhsb[:, ko, ts],
                        start=(ko == 0),
                        stop=(ko == KO - 1),
                    )


            for g in range(NGT):
                gs = slice(g * GT, (g + 1) * GT)
                lgs = gp.tile([E, GT], f32, tag="lgs")
                nc.vector.tensor_copy(lgs[:], lgs_t[g][:])

                # transpose logits to token-major: [tok, chunk, expert]
                ltk = gp.tile([P, NGC, E], f32, tag="ltk")
                for c in range(NGC):
                    tp = ps_b.tile([P, GT], f32, tag="pb", name="tp")[:, :E]
                    nc.tensor.transpose(
                        tp[:], lgs[:, c * P : (c + 1) * P], idsb[:E, :E]
                    )
                    nc.vector.tensor_copy(ltk[:, c, :], tp[:])

                # top-2 + softmax along the free (expert) axis.
                m1 = gp.tile([P, NGC, 1], f32, tag="m1")
                nc.vector.tensor_reduce(m1[:], ltk[:], mybir.AxisListType.X, Alu.max)
                mask1 = gp.tile([P, NGC, E], f32, tag="mask1")
                nc.vector.tensor_tensor(
                    mask1[:], ltk[:], m1.to_broadcast((P, NGC, E)), Alu.is_equal
                )
                l2 = gp.tile([P, NGC, E], f32, tag="l2")
                nc.vector.scalar_tensor_tensor(
                    l2[:], mask1[:], -1e30, ltk[:], Alu.mult, Alu.add
                )
                m2 = gp.tile([P, NGC, 1], f32, tag="m2")
                nc.vector.tensor_reduce(m2[:], l2[:], mybir.AxisListType.X, Alu.max)
                mask2 = gp.tile([P, NGC, E], f32, tag="mask2")
                nc.vector.tensor_tensor(
                    mask2[:], l2[:], m2.to_broadcast((P, NGC, E)), Alu.is_equal
                )
                dlt = gp.tile([P, NGC, 1], f32, tag="dlt")
                nc.vector.tensor_tensor(dlt[:], m2[:], m1[:], Alu.subtract)
                # logits carry the x64 router-weight scale; undo it
                # before the top-2 softmax
                dlts = gp.tile([P, NGC, 1], f32, tag="dlts")
                nc.vector.tensor_scalar(
                    dlts[:], dlt[:], 1.0 / 64.0, 0.0, Alu.mult, Alu.add
                )
                g2 = gp.tile([P, NGC, 1], f32, tag="g2")
                nc.scalar.activation(g2[:], dlts[:], Act.Sigmoid)
                g1 = gp.tile([P, NGC, 1], f32, tag="g1")
                nc.vector.tensor_scalar(g1[:], g2[:], -1.0, 1.0, Alu.mult, Alu.add)

                gate = gp.tile([P, NGC, E], f32, tag="gate")
                nc.vector.tensor_tensor(
                    gate[:], mask1[:], g1.to_broadcast((P, NGC, E)), Alu.mult
                )
                gm2 = gp.tile([P, NGC, E], f32, tag="gm2")
                nc.vector.tensor_tensor(
                    gm2[:], mask2[:], g2.to_broadcast((P, NGC, E)), Alu.mult
                )
                nc.vector.tensor_tensor(gate[:], gate[:], gm2[:], Alu.add)

                # transpose gates back to expert-major [8, 256]
                gts = gp.tile([E, GT], f32, tag="gts")
                for c in range(NGC):
                    tp2 = ps_b.tile([P, GT], f32, tag="pb", name="tp2")[:E, :P]
                    nc.tensor.transpose(tp2[:], gate[:, c, :], idsb[:])
                    nc.vector.tensor_copy(gts[:, c * P : (c + 1) * P], tp2[:])

                # expand expert gates (x scaling, folded into e8) to the
                # 128 rank slots: RG = e8.T @ gts
                RG = ps_b.tile([P, GT], f32, tag="pb", name="RG")
                nc.tensor.matmul(
                    RG[:], lhsT=e8sb[:], rhs=gts[:], start=True, stop=True
                )
                nc.vector.tensor_copy(rgp[:, gs], RG[:])

            # gated rank activations (x64-scaled when FP8_BASE: the x64 is
            # folded into e8 -> rgp, cancelling the fp8 weight scale)
            for tt in range(NT):
                ts = slice(tt * TTILE, (tt + 1) * TTILE)
                nc.vector.tensor_tensor(
                    hwsb[:, ts], h_t[tt][:], rgp[:, ts], Alu.mult
                )

            phase1.close()

            # ---- phase 2: base matmul + fused lora_B ----
            phase2 = ExitStack()
            ps_o = phase2.enter_context(
                tc.tile_pool(name="ps_o", bufs=7, space="PSUM")
            )

            for ot in range(OTILES):
                os_ = slice(ot * P, (ot + 1) * P)
                if FP8_BASE:
                    if ot < len(pre_w):
                        wsb = pre_w[ot]
                    else:
                        wsb = wpool.tile([P, KO2, 2, P], f8, tag="w")
                        nc.gpsimd.dma_start(wsb[:], wt_v[:, ot, :, :, :])
                    for tt in range(NT):
                        ts = slice(tt * TTILE, (tt + 1) * TTILE)
                        acc = ps_o.tile([P, TTILE], f32, tag="acc")
                        for k2 in range(KO2):
                            nc.tensor.matmul(
                                acc[:],
                                lhsT=wsb[:, k2, :, :],
                                rhs=x8sb[:, 2 * k2 : 2 * k2 + 2, ts],
                                start=(k2 == 0),
                                stop=False,
                                perf_mode=DR,
                            )
                        nc.tensor.matmul(
                            acc[:],
                            lhsT=lbsb[:, os_],
                            rhs=hwsb[:, ts],
                            start=False,
                            stop=True,
                        )
                        osb = ob.tile([P, TTILE], f32, tag="osb")
                        # acc holds 64x(base+lora); rescale + bias in one op
                        nc.vector.scalar_tensor_tensor(
                            osb[:],
                            acc[:],
                            1.0 / W8_SCALE,
                            bbsb[:, ot, None].to_broadcast((P, TTILE)),
                            Alu.mult,
                            Alu.add,
                        )
                        nc.sync.dma_start(yt_v[:, ot, ts], osb[:])
                else:
                    wsb = wpool.tile([P, KO_EXT, P], wdt, tag="w")
                    nc.gpsimd.dma_start(wsb[:], wt_v[:, ot, :, :])
                    for tt in range(NT):
                        ts = slice(tt * TTILE, (tt + 1) * TTILE)
                        acc = ps_o.tile([P, TTILE], f32, tag="acc")
                        for ko in range(KO):
                            nc.tensor.matmul(
                                acc[:],
                                lhsT=wsb[:, ko, :],
                                rhs=xsb[:, ko, ts],
                                start=(ko == 0),
                                stop=False,
                            )
                        nc.tensor.matmul(
                            acc[:],
                            lhsT=wsb[:, KO, :],
                            rhs=hwsb[:, ts],
                            start=False,
                            stop=True,
                        )
                        osb = ob.tile([P, TTILE], f32, tag="osb")
                        nc.vector.tensor_tensor(
                            osb[:],
                            acc[:],
                            bbsb[:, ot, None].to_broadcast((P, TTILE)),
                            Alu.add,
                        )
                        nc.sync.dma_start(yt_v[:, ot, ts], osb[:])
            phase2.close()

    nc.compile()
    return nc


def get_program():
    if "nc" not in _prog_cache:
        _prog_cache["nc"] = _build_program()
    return _prog_cache["nc"]


def make_in_maps(x, base_w, base_b, lora_A, lora_B, router_w, scalings):
    """Host-side sharding/layout prep -> per-core input dicts."""
    import ml_dtypes

    wnp = ml_dtypes.bfloat16 if WT_BF16 else np.float32

    x = np.ascontiguousarray(x, dtype=np.float32)
    xt_full = np.ascontiguousarray(x.reshape(T, D).T)  # [D, T]

    lb_host = None
    if FP8_BASE:
        # base weights x64 -> e4m3, DoubleRow pair layout [ot,p,k2,j,f]
        wt_host = np.ascontiguousarray(
            (base_w.T.astype(np.float32) * W8_SCALE)
            .reshape(KO2, 2, P, OTILES, P)
            .transpose(3, 2, 0, 1, 4)
            .reshape(OTILES * P, KO2 * 2 * P)
            .astype(ml_dtypes.float8_e4m3)
        )
        lb_host = np.ascontiguousarray(lora_B.T.astype(np.float32).astype(wnp))
    else:
        # W_ext = [base_w.T ; lora_B.T]  ->  [ot, p, ko*128+f] layout
        w_ext = np.empty((KO_EXT * P, O), dtype=np.float32)
        w_ext[:D] = base_w.T
        w_ext[D:] = lora_B.T
        wt_host = np.ascontiguousarray(
            w_ext.reshape(KO_EXT, P, OTILES, P)
            .transpose(2, 1, 0, 3)
            .reshape(OTILES * P, KO_EXT * P)
            .astype(wnp)
        )

    # lora_A.T (unscaled; scaling folded into e8) -> [p, ko*128+r]
    ar_host = np.ascontiguousarray(
        lora_A.T.astype(np.float32)
        .reshape(KO, P, R)
        .transpose(1, 0, 2)
        .reshape(P, KO * R)
        .astype(np.float16)
    )

    # router_w.T x64 -> [p, ko*8+e], fp16 hi/lo split (hi + lo == 64*rw
    # to ~2^-24 relative; x64 keeps the lo part in fp16 normal range)
    rw64 = np.ascontiguousarray(
        router_w.T.astype(np.float32)
        .reshape(KO, P, E)
        .transpose(1, 0, 2)
        .reshape(P, KO * E)
    ) * np.float32(64.0)
    rwh_host = rw64.astype(np.float16)
    rwr_host = (rw64 - rwh_host.astype(np.float32)).astype(np.float16)

    # expert -> rank-slot expansion with per-expert scaling folded in;
    # when FP8_BASE the x64 weight scale is folded here too so the lora
    # matmul accumulates at the same scale as the fp8 base steps
    e8 = np.zeros((E, P), dtype=np.float32)
    s = np.asarray(scalings, dtype=np.float32)
    if FP8_BASE:
        s = s * W8_SCALE
    for e in range(E):
        e8[e, e * RANK : (e + 1) * RANK] = s[e]
    idm = np.eye(P, dtype=np.float32)
    bbf = base_b.astype(np.float32)

    xh_full = xt_full.astype(np.float16)
    rh_full = (xt_full - xh_full.astype(np.float32)).astype(np.float16)
    x8_full = xt_full.astype(ml_dtypes.float8_e4m3)

    in_maps = []
    for c in range(N_CORES):
        cs = slice(c * T_PC, (c + 1) * T_PC)
        m = {
            "xh": np.ascontiguousarray(xh_full[:, cs]),
            "rh": np.ascontiguousarray(rh_full[:, cs]),
            "x8": np.ascontiguousarray(x8_full[:, cs]),
            "wt": wt_host,
            "ar": ar_host,
            "rwh": rwh_host,
            "rwr": rwr_host,
            "bb": bbf,
            "e8": e8,
            "idm": idm,
        }
        if FP8_BASE:
            m["lb"] = lb_host
        in_maps.append(m)
    return in_maps


def assemble_output(results):
    """Per-core yt [O, T_PC] -> full [B, S, O]."""
    yt_full = np.concatenate([r["yt"] for r in results], axis=1)  # [O, T]
    return np.ascontiguousarray(yt_full.T).reshape(B, S, O)


def kernel(**inputs):
    _ensure_path()
    from concourse.bass_utils import run_bass_kernel_spmd

    assert int(inputs["top_k"]) == 2
    nc = get_program()
    in_maps = make_in_maps(
        inputs["x"],
        inputs["base_w"],
        inputs["base_b"],
        inputs["lora_A"],
        inputs["lora_B"],
        inputs["router_w"],
        inputs["scalings"],
    )
    res = run_bass_kernel_spmd(nc, in_maps, list(range(N_CORES)))
    return assemble_output(res.results)


if __name__ == "__main__":
    # quick smoke: build the program only
    get_program()
    print("program built OK")



# revision 12
# speedup vs baseline: 1.0483x; 1.0483x over previous
"""Trainium2 Bass kernel for nn_GatedLinear (gated LoRA-MoE linear layer).

Math (see reference):
  base_out = x @ base_w.T + base_b
  logits   = x @ router_w.T ; top-2 softmax -> dense per-expert gate
  h        = x @ lora_A.T   ; rank_w = repeat(gate*scalings, 16)
  out      = base_out + (h * rank_w) @ lora_B.T

Sharding: pure data-parallel over batch*seq across 8 cores (1024 tokens
per core); all weights replicated. No collectives.

Device-side strategy (v6):
  * x ships ONLY as an fp16 hi/lo split (xh + rh == x to ~2^-23 rel):
    16MB/core, the minimum for a top-2 selection that matches the fp32
    reference. The fp8 copy for the base matmul is CAST ON DEVICE from
    xh by the (otherwise idle) vector engine -- 4MB less DMA in the
    phase-1 critical window than shipping it.
  * Router: the two xh terms (xh@rwh, xh@rwr) merge into ONE matmul
    with a 16-wide stationary [rwh|rwr]; the rh term (rh@rwh)
    accumulates into rows 0:8 of the same PSUM tile; after the close a
    single DVE add folds rows 8:16 into rows 0:8. 2 matmuls/(ko,g)
    instead of 3 -- saves ~14us of PE.
  * Base matmul: fp8e4m3 DoubleRow (weights host-scaled x64, packed
    [k2,2,f]); measured 216ns per 256x128x512 step = the fp8 roofline.
    The x64 scale is folded into the e8 gate expansion and removed in
    the bias epilogue.
  * Two "early" base groups (ot=0) run during phase-1 streaming to
    keep the PE dense while xh/rh land; their lora_B closes wait for
    the gated rank activations and are emitted after the gating chain.
  * DMA queues: xh striped even/odd over sync/vector; rh as an 8-deep
    ring on gpsimd (each chunk is dead after its router term); weights
    split scalar/gpsimd, first tile prefetched ahead of the stream;
    fp16 outputs on sync.
  * Output is fp16 [O, tokens] per core (halves output DMA; ~5e-4 rel
    error) and de-transposed/cast to f32 on the host.

PSUM budget during phase 1 (8 banks): 2 logits + 2 h + 2 transpose
scratch + 2 early base groups. Phase 2 reuses all 8 as 6 accumulation
groups + scratch.
"""

from contextlib import ExitStack

import numpy as np


def _ensure_path():
    try:
        import concourse.bass  # noqa: F401
    except ImportError:
        import sys

        for p in ("/opt/trn_rl_repo", "/root/.axon_site/_ro/trn_rl_repo"):
            if p not in sys.path:
                sys.path.insert(0, p)


N_CORES = 8
B, S, D, O = 4, 2048, 4096, 4096
T = B * S              # 8192 tokens total
T_PC = T // N_CORES    # 1024 tokens per core
E = 8                  # experts
RANK = 16
R = E * RANK           # 128 fused rank dim
P = 128
KO = D // P            # 32 k-subtiles of the contraction dim
KO2 = KO // 2          # paired k-subtiles for DoubleRow (256-deep)
OTILES = O // P        # 32 output-feature tiles
TTILE = 512            # tokens per matmul moving operand
NT = T_PC // TTILE     # 2 token tiles per core
GT = 512               # gating token-tile size
NGT = T_PC // GT       # 2 gating tiles
NGC = GT // P          # 4 128-chunks per gating tile
W8_SCALE = 64.0        # base_w std is 1/64; scale into e4m3's sweet spot
FP8_BASE = True        # kept for test.py's sim threshold selection

_prog_cache = {}


def _build_program():
    """Build the single-core SPMD Bass program (same on all 8 cores)."""
    _ensure_path()
    import concourse.bass as bass
    import concourse.mybir as mybir
    import concourse.tile as tile
    from concourse import bacc

    f32 = mybir.dt.float32
    f16 = mybir.dt.float16
    bf16 = mybir.dt.bfloat16
    f8 = mybir.dt.float8e4
    Alu = mybir.AluOpType
    Act = mybir.ActivationFunctionType
    DR = mybir.MatmulPerfMode.DoubleRow

    nc = bacc.Bacc(
        "TRN2",
        target_bir_lowering=False,
        debug=False,
        num_devices=N_CORES,
    )

    # x hi/lo ship partition-major so a 4-ko block is one 1MB DMA with
    # 8KB contiguous runs per partition (~341 GB/s vs ~100 for 2KB runs)
    xh = nc.dram_tensor("xh", [P, KO * T_PC], f16, kind="ExternalInput").ap()
    rh = nc.dram_tensor("rh", [P, KO * T_PC], f16, kind="ExternalInput").ap()
    wt = nc.dram_tensor(
        "wt", [OTILES * P, KO2 * 2 * P], f8, kind="ExternalInput"
    ).ap()
    lb = nc.dram_tensor("lb", [P, O], bf16, kind="ExternalInput").ap()
    ar = nc.dram_tensor("ar", [P, KO * R], f16, kind="ExternalInput").ap()
    rw2 = nc.dram_tensor("rw2", [P, KO * 2 * E], f16, kind="ExternalInput").ap()
    bb = nc.dram_tensor("bb", [O], f32, kind="ExternalInput").ap()
    e8 = nc.dram_tensor("e8", [E, P], f32, kind="ExternalInput").ap()
    idm = nc.dram_tensor("idm", [P, P], f32, kind="ExternalInput").ap()
    yt = nc.dram_tensor("yt", [O, T_PC], f16, kind="ExternalOutput").ap()

    xh_v = xh.rearrange("p (ko t) -> p ko t", t=T_PC)
    rh_v = rh.rearrange("p (ko t) -> p ko t", t=T_PC)
    wt_v = wt.rearrange("(ot p) (k j f) -> p ot k j f", p=P, j=2, f=P)
    ar_v = ar.rearrange("p (ko r) -> p ko r", r=R)          # [128, 32, 128]
    rw2_v = rw2.rearrange("p (ko c) -> p ko c", c=2 * E)    # [128, 32, 16]
    bb_v = bb.rearrange("(ot p) -> p ot", p=P)              # [128, 32]
    yt_v = yt.rearrange("(ot p) t -> p ot t", p=P)          # [128, 32, 1024]

    with tile.TileContext(nc) as tc:
        with (
            tc.tile_pool(name="perm", bufs=1) as pp,
            tc.tile_pool(name="wstream", bufs=6) as wpool,
            tc.tile_pool(name="rring", bufs=3) as rpool,
            tc.tile_pool(name="obuf", bufs=6) as ob,
        ):
            # ---- consts + resident tiles ----
            # scalar queue: rw2 (needed by the very first matmul) then
            # the rh block stream
            rw2sb = pp.tile([P, KO, 2 * E], f16)
            nc.scalar.dma_start(rw2sb[:], rw2_v[:])

            # gpsimd queue: lora_A + gating consts + first weight tiles
            arsb = pp.tile([P, KO, R], f16)
            nc.gpsimd.dma_start(arsb[:], ar_v[:])
            e8sb = pp.tile([E, P], f32)
            nc.gpsimd.dma_start(e8sb[:], e8[:])
            idsb = pp.tile([P, P], f32)
            nc.gpsimd.dma_start(idsb[:], idm[:])
            w_sb = [None] * OTILES
            for ot in (0, 1, 2, 3):
                w_sb[ot] = wpool.tile([P, KO2, 2, P], f8, tag="w", name=f"w{ot}")
                nc.gpsimd.dma_start(w_sb[ot][:], wt_v[:, ot, :, :, :])

            # resident fp16 hi copy of x + on-device fp8 cast target
            xhsb = pp.tile([P, KO, T_PC], f16)
            x8sb = pp.tile([P, KO, T_PC], f8)
            rgp = pp.tile([P, T_PC], bf16)   # per-rank gates [r, t]
            hwsb = pp.tile([P, T_PC], bf16)  # gated rank activations [r, t]
            lbsb = pp.tile([P, O], bf16)     # lora_B.T resident
            bbsb = pp.tile([P, OTILES], f32)

            # xh: 8 x 1MB 4-ko blocks on sync (HWDGE)
            KB = 4                   # kos per x block
            NB = KO // KB            # 8 blocks
            for b in range(NB):
                nc.sync.dma_start(
                    xhsb[:, b * KB : (b + 1) * KB, :],
                    xh_v[:, b * KB : (b + 1) * KB, :],
                )
            # rh: ring of 4-ko block tiles on scalar; each block is dead
            # once its router terms ran, so a 3-deep ring is plenty
            rh_t = []
            for b in range(NB):
                t_ = rpool.tile([P, KB, T_PC], f16, tag="rh", name=f"rh{b}")
                nc.scalar.dma_start(t_[:], rh_v[:, b * KB : (b + 1) * KB, :])
                rh_t.append(t_)
            # lora_B / bias / one more weight tile behind rh on scalar
            nc.gpsimd.dma_start(lbsb[:], lb[:])
            nc.gpsimd.dma_start(bbsb[:], bb_v[:])
            w_sb[4] = wpool.tile([P, KO2, 2, P], f8, tag="w", name="w4")
            nc.scalar.dma_start(w_sb[4][:], wt_v[:, 4, :, :, :])

            # ---- phase 1: router + h + early base, paced by x arrival ----
            phase1 = ExitStack()
            gp = phase1.enter_context(tc.tile_pool(name="gtmp", bufs=1))
            ps_l = phase1.enter_context(
                tc.tile_pool(name="ps_l", bufs=NGT, space="PSUM")
            )
            ps_h = phase1.enter_context(
                tc.tile_pool(name="ps_h", bufs=2, space="PSUM")
            )
            ps_t = phase1.enter_context(
                tc.tile_pool(name="ps_t", bufs=2, space="PSUM")
            )
            ps_e = phase1.enter_context(
                tc.tile_pool(name="ps_e", bufs=2, space="PSUM")
            )

            # logits PSUM [16, GT]: rows 0:8 = xh@rwh (+ rh@rwh), rows
            # 8:16 = xh@rwr; folded by one DVE add after the close
            lgs_t = [
                ps_l.tile([2 * E, GT], f32, tag="lg", name=f"lg{g}")
                for g in range(NGT)
            ]
            h_t = [
                ps_h.tile([P, TTILE], f32, tag="h", name=f"h{t}")
                for t in range(NT)
            ]
            # early base groups: (ot=0, tt=0) and (ot=0, tt=1)
            acc_e = [
                ps_e.tile([P, TTILE], f32, tag="acce", name=f"acce{t}")
                for t in range(2)
            ]

            for ko in range(KO):
                for g in range(NGT):
                    gs = slice(g * GT, (g + 1) * GT)
                    # t13 first at ko=0 (start must cover all 16 rows),
                    # last at ko=31 (stop likewise); t2 sandwiched
                    def t13(start, stop):
                        nc.tensor.matmul(
                            lgs_t[g][:],
                            lhsT=rw2sb[:, ko, :],
                            rhs=xhsb[:, ko, gs],
                            start=start,
                            stop=stop,
                        )

                    def t2():
                        nc.tensor.matmul(
                            lgs_t[g][:E, :],
                            lhsT=rw2sb[:, ko, :E],
                            rhs=rh_t[ko // KB][:, ko % KB, gs],
                            start=False,
                            stop=False,
                        )

                    if ko == 0:
                        t13(True, False)
                        t2()
                    elif ko == KO - 1:
                        t2()
                        t13(False, True)
                    else:
                        t2()
                        t13(False, False)
                for tt in range(NT):
                    ts = slice(tt * TTILE, (tt + 1) * TTILE)
                    nc.tensor.matmul(
                        h_t[tt][:],
                        lhsT=arsb[:, ko, :],
                        rhs=xhsb[:, ko, ts],
                        start=(ko == 0),
                        stop=(ko == KO - 1),
                    )
                # fp8 cast of this chunk (vector engine, off critical path)
                nc.vector.tensor_copy(x8sb[:, ko, :], xhsb[:, ko, :])
                # early base DR step after each odd chunk's cast
                if ko % 2 == 1:
                    k2 = ko // 2
                    for tt in range(2):
                        ts = slice(tt * TTILE, (tt + 1) * TTILE)
                        nc.tensor.matmul(
                            acc_e[tt][:],
                            lhsT=w_sb[0][:, k2, :, :],
                            rhs=x8sb[:, 2 * k2 : 2 * k2 + 2, ts],
                            start=(k2 == 0),
                            stop=False,
                            perf_mode=DR,
                        )

            # ---- gating: top-2 softmax -> per-rank gates ----
            for g in range(NGT):
                gs = slice(g * GT, (g + 1) * GT)
                # copy the [16, GT] logits PSUM to SBUF, transpose to
                # token-major, then fold the rwh/rwr halves along the
                # FREE axis (cross-partition reads at offset 8 are
                # rejected by the BIR verifier)
                lgs16 = gp.tile([2 * E, GT], f32, tag="lgs", name=f"lgs{g}")
                nc.vector.tensor_copy(lgs16[:], lgs_t[g][:])
                ltk16 = gp.tile([P, NGC, 2 * E], f32, tag="ltk16", name=f"lt16{g}")
                for c in range(NGC):
                    tp = ps_t.tile([P, GT], f32, tag="pt", name="tp")[:, : 2 * E]
                    nc.tensor.transpose(
                        tp[:], lgs16[:, c * P : (c + 1) * P], idsb[: 2 * E, : 2 * E]
                    )
                    nc.vector.tensor_copy(ltk16[:, c, :], tp[:])
                ltk = gp.tile([P, NGC, E], f32, tag="ltk", name=f"ltk{g}")
                nc.vector.tensor_tensor(
                    ltk[:], ltk16[:, :, :E], ltk16[:, :, E:], Alu.add
                )

                # top-2 + softmax along the free (expert) axis
                m1 = gp.tile([P, NGC, 1], f32, tag="m1")
                nc.vector.tensor_reduce(m1[:], ltk[:], mybir.AxisListType.X, Alu.max)
                mask1 = gp.tile([P, NGC, E], f32, tag="mask1")
                nc.vector.tensor_tensor(
                    mask1[:], ltk[:], m1.to_broadcast((P, NGC, E)), Alu.is_equal
                )
                l2 = gp.tile([P, NGC, E], f32, tag="l2")
                nc.vector.scalar_tensor_tensor(
                    l2[:], mask1[:], -1e30, ltk[:], Alu.mult, Alu.add
                )
                m2 = gp.tile([P, NGC, 1], f32, tag="m2")
                nc.vector.tensor_reduce(m2[:], l2[:], mybir.AxisListType.X, Alu.max)
                mask2 = gp.tile([P, NGC, E], f32, tag="mask2")
                nc.vector.tensor_tensor(
                    mask2[:], l2[:], m2.to_broadcast((P, NGC, E)), Alu.is_equal
                )
                dlt = gp.tile([P, NGC, 1], f32, tag="dlt")
                nc.vector.tensor_tensor(dlt[:], m2[:], m1[:], Alu.subtract)
                # logits carry the x64 router-weight scale; undo it
                dlts = gp.tile([P, NGC, 1], f32, tag="dlts")
                nc.vector.tensor_scalar(
                    dlts[:], dlt[:], 1.0 / 64.0, 0.0, Alu.mult, Alu.add
                )
                g2 = gp.tile([P, NGC, 1], f32, tag="g2")
                nc.scalar.activation(g2[:], dlts[:], Act.Sigmoid)
                g1 = gp.tile([P, NGC, 1], f32, tag="g1")
                nc.vector.tensor_scalar(g1[:], g2[:], -1.0, 1.0, Alu.mult, Alu.add)

                gate = gp.tile([P, NGC, E], f32, tag="gate")
                nc.vector.tensor_tensor(
                    gate[:], mask1[:], g1.to_broadcast((P, NGC, E)), Alu.mult
                )
                gm2 = gp.tile([P, NGC, E], f32, tag="gm2")
                nc.vector.tensor_tensor(
                    gm2[:], mask2[:], g2.to_broadcast((P, NGC, E)), Alu.mult
                )
                nc.vector.tensor_tensor(gate[:], gate[:], gm2[:], Alu.add)

                # transpose gates back to expert-major [8, 512]
                gts = gp.tile([E, GT], f32, tag="gts", name=f"gts{g}")
                for c in range(NGC):
                    tp2 = ps_t.tile([P, GT], f32, tag="pt", name="tp2")[:E, :P]
                    nc.tensor.transpose(tp2[:], gate[:, c, :], idsb[:])
                    nc.vector.tensor_copy(gts[:, c * P : (c + 1) * P], tp2[:])

                # expand expert gates (x scaling*64, folded into e8) to
                # the 128 rank slots: RG = e8.T @ gts
                RG = ps_t.tile([P, GT], f32, tag="pt", name="RG")
                nc.tensor.matmul(
                    RG[:], lhsT=e8sb[:], rhs=gts[:], start=True, stop=True
                )
                nc.vector.tensor_copy(rgp[:, gs], RG[:])
                # gated rank activations for this token tile (g == tt)
                nc.vector.tensor_tensor(
                    hwsb[:, gs], h_t[g][:], rgp[:, gs], Alu.mult
                )

            # ---- close the early groups: lora term + epilogue ----
            for tt in range(2):
                ts = slice(tt * TTILE, (tt + 1) * TTILE)
                nc.tensor.matmul(
                    acc_e[tt][:],
                    lhsT=lbsb[:, 0:P],
                    rhs=hwsb[:, ts],
                    start=False,
                    stop=True,
                )
                osb = ob.tile([P, TTILE], f16, tag="osb")
                nc.vector.scalar_tensor_tensor(
                    osb[:],
                    acc_e[tt][:],
                    1.0 / W8_SCALE,
                    bbsb[:, 0, None].to_broadcast((P, TTILE)),
                    Alu.mult,
                    Alu.add,
                )
                nc.sync.dma_start(yt_v[:, 0, ts], osb[:])

            phase1.close()

            # ---- phase 2: remaining base matmul + fused lora_B ----
            phase2 = ExitStack()
            ps_o = phase2.enter_context(
                tc.tile_pool(name="ps_o", bufs=6, space="PSUM")
            )

            for ot in range(1, OTILES):
                # keep the weight stream ~4 tiles ahead, alternating queues
                pre = ot + 4
                if pre < OTILES and w_sb[pre] is None:
                    w_sb[pre] = wpool.tile([P, KO2, 2, P], f8, tag="w", name=f"w{pre}")
                    eng = nc.scalar if pre % 2 == 0 else nc.gpsimd
                    eng.dma_start(w_sb[pre][:], wt_v[:, pre, :, :, :])
                os_ = slice(ot * P, (ot + 1) * P)
                for tt in range(NT):
                    ts = slice(tt * TTILE, (tt + 1) * TTILE)
                    acc = ps_o.tile([P, TTILE], f32, tag="acc")
                    for k2 in range(KO2):
                        nc.tensor.matmul(
                            acc[:],
                            lhsT=w_sb[ot][:, k2, :, :],
                            rhs=x8sb[:, 2 * k2 : 2 * k2 + 2, ts],
                            start=(k2 == 0),
                            stop=False,
                            perf_mode=DR,
                        )
                    nc.tensor.matmul(
                        acc[:],
                        lhsT=lbsb[:, os_],
                        rhs=hwsb[:, ts],
                        start=False,
                        stop=True,
                    )
                    osb = ob.tile([P, TTILE], f16, tag="osb")
                    # acc holds 64x(base+lora); rescale + bias in one op
                    nc.vector.scalar_tensor_tensor(
                        osb[:],
                        acc[:],
                        1.0 / W8_SCALE,
                        bbsb[:, ot, None].to_broadcast((P, TTILE)),
                        Alu.mult,
                        Alu.add,
                    )
                    nc.sync.dma_start(yt_v[:, ot, ts], osb[:])
            phase2.close()

    nc.compile()
    return nc


def get_program():
    if "nc" not in _prog_cache:
        _prog_cache["nc"] = _build_program()
    return _prog_cache["nc"]


def make_in_maps(x, base_w, base_b, lora_A, lora_B, router_w, scalings):
    """Host-side sharding/layout prep -> per-core input dicts."""
    import ml_dtypes

    x = np.ascontiguousarray(x, dtype=np.float32)
    # partition-major layout [P, KO, T]: per-core 4-ko DMA blocks are
    # 1MB with 8KB contiguous runs per partition
    xt_full = np.ascontiguousarray(
        x.reshape(T, KO, P).transpose(2, 1, 0)
    )  # [P, KO, T]

    # base weights x64 -> e4m3, DoubleRow pair layout [ot,p,k2,j,f]
    wt_host = np.ascontiguousarray(
        (base_w.T.astype(np.float32) * W8_SCALE)
        .reshape(KO2, 2, P, OTILES, P)
        .transpose(3, 2, 0, 1, 4)
        .reshape(OTILES * P, KO2 * 2 * P)
        .astype(ml_dtypes.float8_e4m3)
    )
    lb_host = np.ascontiguousarray(
        lora_B.T.astype(np.float32).astype(ml_dtypes.bfloat16)
    )

    # lora_A.T (unscaled; scaling folded into e8) -> [p, ko*128+r]
    ar_host = np.ascontiguousarray(
        lora_A.T.astype(np.float32)
        .reshape(KO, P, R)
        .transpose(1, 0, 2)
        .reshape(P, KO * R)
        .astype(np.float16)
    )

    # router_w.T x64 -> [p, ko, 16]: cols 0:8 = fp16 hi, 8:16 = fp16 lo
    # (hi + lo == 64*rw to ~2^-24 relative)
    rw64 = np.ascontiguousarray(
        router_w.T.astype(np.float32)
        .reshape(KO, P, E)
        .transpose(1, 0, 2)
    ) * np.float32(64.0)                                  # [P, KO, E]
    rwh_host = rw64.astype(np.float16)
    rwr_host = (rw64 - rwh_host.astype(np.float32)).astype(np.float16)
    rw2_host = np.ascontiguousarray(
        np.concatenate([rwh_host, rwr_host], axis=-1).reshape(P, KO * 2 * E)
    )

    # expert -> rank-slot expansion with per-expert scaling and the x64
    # fp8 weight scale folded in (so the lora matmul accumulates at the
    # same scale as the fp8 base steps)
    e8 = np.zeros((E, P), dtype=np.float32)
    s = np.asarray(scalings, dtype=np.float32) * W8_SCALE
    for e in range(E):
        e8[e, e * RANK : (e + 1) * RANK] = s[e]
    idm = np.eye(P, dtype=np.float32)
    bbf = base_b.astype(np.float32)

    xh_full = xt_full.astype(np.float16)
    rh_full = (xt_full - xh_full.astype(np.float32)).astype(np.float16)

    in_maps = []
    for c in range(N_CORES):
        cs = slice(c * T_PC, (c + 1) * T_PC)
        m = {
            "xh": np.ascontiguousarray(xh_full[:, :, cs]).reshape(P, KO * T_PC),
            "rh": np.ascontiguousarray(rh_full[:, :, cs]).reshape(P, KO * T_PC),
            "wt": wt_host,
            "lb": lb_host,
            "ar": ar_host,
            "rw2": rw2_host,
            "bb": bbf,
            "e8": e8,
            "idm": idm,
        }
        in_maps.append(m)
    return in_maps


def assemble_output(results):
    """Per-core yt [O, T_PC] fp16 -> full [B, S, O] f32."""
    yt_full = np.concatenate(
        [np.asarray(r["yt"]) for r in results], axis=1
    )  # [O, T] fp16
    return np.ascontiguousarray(yt_full.T.astype(np.float32)).reshape(B, S, O)


def kernel(**inputs):
    _ensure_path()
    from concourse.bass_utils import run_bass_kernel_spmd

    assert int(inputs["top_k"]) == 2
    nc = get_program()
    in_maps = make_in_maps(
        inputs["x"],
        inputs["base_w"],
        inputs["base_b"],
        inputs["lora_A"],
        inputs["lora_B"],
        inputs["router_w"],
        inputs["scalings"],
    )
    res = run_bass_kernel_spmd(nc, in_maps, list(range(N_CORES)))
    return assemble_output(res.results)


if __name__ == "__main__":
    # quick smoke: build the program only
    get_program()
    print("program built OK")


# revision 15
# speedup vs baseline: 1.0575x; 1.0088x over previous
"""Trainium2 Bass kernel for nn_GatedLinear (gated LoRA-MoE linear layer).

Math (see reference):
  base_out = x @ base_w.T + base_b
  logits   = x @ router_w.T ; top-2 softmax -> dense per-expert gate
  h        = x @ lora_A.T   ; rank_w = repeat(gate*scalings, 16)
  out      = base_out + (h * rank_w) @ lora_B.T

Sharding: pure data-parallel over batch*seq across 8 cores (1024 tokens
per core); all weights replicated. No collectives.

Device-side strategy (v7):
  * x ships ONLY as an fp16 hi/lo split (xh + rh == x to ~2^-23 rel):
    16MB/core, the minimum for a top-2 selection that matches the fp32
    reference; partition-major layout so a 4-ko block is one 1MB DMA
    with 8KB contiguous runs. The fp8 copy for the base matmul is cast
    on device from xh by the scalar (ACT) engine -- 4MB less DMA in
    the phase-1 critical window than shipping it.
  * Router: the two xh terms (xh@rwh, xh@rwr) merge into ONE matmul
    with a 16-wide stationary [rwh|rwr]; the rh term (rh@rwh)
    accumulates into rows 0:8 of the same [16,GT] PSUM tile; the fold
    happens after the token-major transpose along the free axis (the
    BIR verifier rejects partition-offset PSUM reads).
  * Base matmul: fp8e4m3 DoubleRow (weights host-scaled x64, packed
    [k2,2,f]); measured 216ns per 256x128x512 step = the fp8 roofline
    (HAM-warm 2.4GHz, 1 cycle/moving-token). The x64 scale is folded
    into the e8 gate expansion and removed in the bias epilogue.
  * Four "early" base groups: ot=0 runs during phase-1 streaming
    (interleaved per-ko); ot=1 fills the PE while the DVE runs the
    gating chain (its PSUM banks come from the freed logits tiles).
    All four lora_B closes are emitted after the gating chain.
  * DMA queues: per-queue HBM share is ~1/3 of 358 GB/s when all three
    queues are loaded, so xh/rh blocks ALTERNATE between sync and
    gpsimd (neither stream bound by one queue's share); scalar's queue
    carries only small consts so its engine is free to cast; weight
    stream split scalar/gpsimd behind phase 1; fp16 outputs on sync.
  * Output is fp16 [O, tokens] per core (halves output DMA; ~5e-4 rel
    error) and de-transposed/cast to f32 on the host.

PSUM budget during phase 1 (8 banks): 2 logits + 2 h + 2 transpose
scratch + 2 early base groups; the logits banks recycle into 2 more
base groups mid-gating. Phase 2 uses 6 accumulation groups.
"""

from contextlib import ExitStack

import numpy as np


def _ensure_path():
    try:
        import concourse.bass  # noqa: F401
    except ImportError:
        import sys

        for p in ("/opt/trn_rl_repo", "/root/.axon_site/_ro/trn_rl_repo"):
            if p not in sys.path:
                sys.path.insert(0, p)


N_CORES = 8
B, S, D, O = 4, 2048, 4096, 4096
T = B * S              # 8192 tokens total
T_PC = T // N_CORES    # 1024 tokens per core
E = 8                  # experts
RANK = 16
R = E * RANK           # 128 fused rank dim
P = 128
KO = D // P            # 32 k-subtiles of the contraction dim
KO2 = KO // 2          # paired k-subtiles for DoubleRow (256-deep)
OTILES = O // P        # 32 output-feature tiles
TTILE = 512            # tokens per matmul moving operand
NT = T_PC // TTILE     # 2 token tiles per core
GT = 512               # gating token-tile size
NGT = T_PC // GT       # 2 gating tiles
NGC = GT // P          # 4 128-chunks per gating tile
W8_SCALE = 64.0        # base_w std is 1/64; scale into e4m3's sweet spot
FP8_BASE = True        # kept for test.py's sim threshold selection

_prog_cache = {}


def _build_program():
    """Build the single-core SPMD Bass program (same on all 8 cores)."""
    _ensure_path()
    import concourse.bass as bass
    import concourse.mybir as mybir
    import concourse.tile as tile
    from concourse import bacc

    f32 = mybir.dt.float32
    f16 = mybir.dt.float16
    bf16 = mybir.dt.bfloat16
    f8 = mybir.dt.float8e4
    Alu = mybir.AluOpType
    Act = mybir.ActivationFunctionType
    DR = mybir.MatmulPerfMode.DoubleRow

    nc = bacc.Bacc(
        "TRN2",
        target_bir_lowering=False,
        debug=False,
        num_devices=N_CORES,
    )

    # x hi/lo ship partition-major so a 4-ko block is one 1MB DMA with
    # 8KB contiguous runs per partition (~341 GB/s vs ~100 for 2KB runs)
    xh = nc.dram_tensor("xh", [P, KO * T_PC], f16, kind="ExternalInput").ap()
    rh = nc.dram_tensor("rh", [P, KO * T_PC], f16, kind="ExternalInput").ap()
    wt = nc.dram_tensor(
        "wt", [OTILES * P, KO2 * 2 * P], f8, kind="ExternalInput"
    ).ap()
    lb = nc.dram_tensor("lb", [P, O], bf16, kind="ExternalInput").ap()
    ar = nc.dram_tensor("ar", [P, KO * R], f16, kind="ExternalInput").ap()
    rw2 = nc.dram_tensor("rw2", [P, KO * 2 * E], f16, kind="ExternalInput").ap()
    bb = nc.dram_tensor("bb", [O], f32, kind="ExternalInput").ap()
    e8 = nc.dram_tensor("e8", [E, P], f32, kind="ExternalInput").ap()
    idm = nc.dram_tensor("idm", [P, P], f32, kind="ExternalInput").ap()
    yt = nc.dram_tensor("yt", [O, T_PC], f16, kind="ExternalOutput").ap()

    xh_v = xh.rearrange("p (ko t) -> p ko t", t=T_PC)
    rh_v = rh.rearrange("p (ko t) -> p ko t", t=T_PC)
    wt_v = wt.rearrange("(ot p) (k j f) -> p ot k j f", p=P, j=2, f=P)
    ar_v = ar.rearrange("p (ko r) -> p ko r", r=R)          # [128, 32, 128]
    rw2_v = rw2.rearrange("p (ko c) -> p ko c", c=2 * E)    # [128, 32, 16]
    bb_v = bb.rearrange("(ot p) -> p ot", p=P)              # [128, 32]
    yt_v = yt.rearrange("(ot p) t -> p ot t", p=P)          # [128, 32, 1024]

    # x block schedule: small leading blocks so the PE starts early,
    # 1MB blocks once streaming; xh/rh alternate between the sync and
    # gpsimd queues so neither stream is bound by one queue's HBM share
    XBLOCKS = [(0, 1), (1, 1), (2, 2), (4, 4), (8, 4), (12, 4), (16, 4),
               (20, 4), (24, 4), (28, 4)]
    KOBLK = {}
    for bi, (s, n) in enumerate(XBLOCKS):
        for k in range(s, s + n):
            KOBLK[k] = (bi, k - s)

    with tile.TileContext(nc) as tc:
        with (
            tc.tile_pool(name="perm", bufs=1) as pp,
            tc.tile_pool(name="wstream", bufs=6) as wpool,
            tc.tile_pool(name="rring", bufs=3) as rpool,
            tc.tile_pool(name="obuf", bufs=6) as ob,
        ):
            # ---- scalar queue (kept nearly empty; its engine does the
            # fp8 casts): rw2, w0, gating consts ----
            rw2sb = pp.tile([P, KO, 2 * E], f16)
            nc.scalar.dma_start(rw2sb[:], rw2_v[:])
            w_sb = [None] * OTILES
            w_sb[0] = wpool.tile([P, KO2, 2, P], f8, tag="w", name="w0")
            nc.scalar.dma_start(w_sb[0][:], wt_v[:, 0, :, :, :])
            e8sb = pp.tile([E, P], f32)
            nc.scalar.dma_start(e8sb[:], e8[:])
            idsb = pp.tile([P, P], f32)
            nc.scalar.dma_start(idsb[:], idm[:])

            # resident tiles
            xhsb = pp.tile([P, KO, T_PC], f16)
            x8sb = pp.tile([P, KO, T_PC], f8)
            rgp = pp.tile([P, T_PC], bf16)   # per-rank gates [r, t]
            hwsb = pp.tile([P, T_PC], bf16)  # gated rank activations [r, t]
            lbsb = pp.tile([P, O], bf16)     # lora_B.T resident
            bbsb = pp.tile([P, OTILES], f32)
            arsb = pp.tile([P, KO, R], f16)

            # lora_A in 4 chunks interleaved into the gpsimd stream
            # (chunk 0 lands before the first h matmul needs it)
            ARC = 8
            nc.gpsimd.dma_start(arsb[:, 0:ARC, :], ar_v[:, 0:ARC, :])

            # x streams: even blocks of xh + odd blocks of rh on sync,
            # the mirror on gpsimd; ar chunks slotted in
            rh_t = [None] * len(XBLOCKS)
            for bi, (s, n) in enumerate(XBLOCKS):
                ksl = slice(s, s + n)
                rh_t[bi] = rpool.tile(
                    [P, n, T_PC], f16, tag=f"rh{n}", name=f"rh{bi}"
                )
                if bi % 2 == 0:
                    nc.sync.dma_start(xhsb[:, ksl, :], xh_v[:, ksl, :])
                    nc.gpsimd.dma_start(rh_t[bi][:], rh_v[:, ksl, :])
                else:
                    nc.sync.dma_start(rh_t[bi][:], rh_v[:, ksl, :])
                    nc.gpsimd.dma_start(xhsb[:, ksl, :], xh_v[:, ksl, :])
                if bi in (3, 5, 7):
                    c = ARC * (bi - 1) // 2
                    nc.gpsimd.dma_start(
                        arsb[:, c : c + ARC, :], ar_v[:, c : c + ARC, :]
                    )
            # tails: lora_B + bias on sync; weight tiles 1-4 on gpsimd
            nc.sync.dma_start(lbsb[:], lb[:])
            nc.sync.dma_start(bbsb[:], bb_v[:])
            for ot in (1, 2, 3, 4):
                w_sb[ot] = wpool.tile([P, KO2, 2, P], f8, tag="w", name=f"w{ot}")
                nc.gpsimd.dma_start(w_sb[ot][:], wt_v[:, ot, :, :, :])

            # ---- phase 1: router + h + early base, paced by x arrival ----
            phase1 = ExitStack()
            stack_l = ExitStack()
            gp = phase1.enter_context(tc.tile_pool(name="gtmp", bufs=2))
            ps_h = phase1.enter_context(
                tc.tile_pool(name="ps_h", bufs=2, space="PSUM")
            )
            ps_t = phase1.enter_context(
                tc.tile_pool(name="ps_t", bufs=2, space="PSUM")
            )
            ps_e = phase1.enter_context(
                tc.tile_pool(name="ps_e", bufs=2, space="PSUM")
            )
            # ps_l created LAST so it can be released first (LIFO),
            # freeing its banks for ps_e2 mid-gating
            ps_l = stack_l.enter_context(
                tc.tile_pool(name="ps_l", bufs=NGT, space="PSUM")
            )

            # logits PSUM [16, GT]: rows 0:8 = xh@rwh (+ rh@rwh), rows
            # 8:16 = xh@rwr; folded after the token-major transpose
            lgs_t = [
                ps_l.tile([2 * E, GT], f32, tag="lg", name=f"lg{g}")
                for g in range(NGT)
            ]
            h_t = [
                ps_h.tile([P, TTILE], f32, tag="h", name=f"h{t}")
                for t in range(NT)
            ]
            # early base groups: (ot=0, tt=0) and (ot=0, tt=1)
            acc_e = [
                ps_e.tile([P, TTILE], f32, tag="acce", name=f"acce{t}")
                for t in range(2)
            ]

            for ko in range(KO):
                bi, off = KOBLK[ko]
                for g in range(NGT):
                    gs = slice(g * GT, (g + 1) * GT)

                    def t13(start, stop):
                        nc.tensor.matmul(
                            lgs_t[g][:],
                            lhsT=rw2sb[:, ko, :],
                            rhs=xhsb[:, ko, gs],
                            start=start,
                            stop=stop,
                        )

                    def t2():
                        nc.tensor.matmul(
                            lgs_t[g][:E, :],
                            lhsT=rw2sb[:, ko, :E],
                            rhs=rh_t[bi][:, off, gs],
                            start=False,
                            stop=False,
                        )

                    if ko == 0:
                        t13(True, False)
                        t2()
                    elif ko == KO - 1:
                        t2()
                        t13(False, True)
                    else:
                        t2()
                        t13(False, False)
                for tt in range(NT):
                    ts = slice(tt * TTILE, (tt + 1) * TTILE)
                    nc.tensor.matmul(
                        h_t[tt][:],
                        lhsT=arsb[:, ko, :],
                        rhs=xhsb[:, ko, ts],
                        start=(ko == 0),
                        stop=(ko == KO - 1),
                    )
                # fp8 cast of this chunk on the scalar engine (its DMA
                # issues are all emitted above, so casts never block them)
                nc.scalar.activation(x8sb[:, ko, :], xhsb[:, ko, :], Act.Copy)
                # early base DR step after each odd chunk's cast
                if ko % 2 == 1:
                    k2 = ko // 2
                    for tt in range(2):
                        ts = slice(tt * TTILE, (tt + 1) * TTILE)
                        nc.tensor.matmul(
                            acc_e[tt][:],
                            lhsT=w_sb[0][:, k2, :, :],
                            rhs=x8sb[:, 2 * k2 : 2 * k2 + 2, ts],
                            start=(k2 == 0),
                            stop=False,
                            perf_mode=DR,
                        )

            # ---- gating: top-2 softmax -> per-rank gates ----
            # copy both logits tiles out of PSUM first, then free their
            # banks for two more base groups (ot=1) that fill the PE
            # while the DVE runs the gating chain
            lgs16 = []
            for g in range(NGT):
                t_ = gp.tile([2 * E, GT], f32, tag="lgs", name=f"lgs{g}")
                nc.vector.tensor_copy(t_[:], lgs_t[g][:])
                lgs16.append(t_)
            stack_l.close()
            ps_e2 = phase1.enter_context(
                tc.tile_pool(name="ps_e2", bufs=2, space="PSUM")
            )
            acc_e2 = [
                ps_e2.tile([P, TTILE], f32, tag="acce2", name=f"acce2{t}")
                for t in range(2)
            ]

            def dr_fill(acc, ot, tt, k2s):
                ts = slice(tt * TTILE, (tt + 1) * TTILE)
                for k2 in k2s:
                    nc.tensor.matmul(
                        acc[:],
                        lhsT=w_sb[ot][:, k2, :, :],
                        rhs=x8sb[:, 2 * k2 : 2 * k2 + 2, ts],
                        start=(k2 == 0),
                        stop=False,
                        perf_mode=DR,
                    )

            # token-major transpose of the [16, GT] logits (both tiles)
            ltk16s = []
            for g in range(NGT):
                ltk16 = gp.tile([P, NGC, 2 * E], f32, tag="ltk16", name=f"lt16{g}")
                for c in range(NGC):
                    tp = ps_t.tile([P, GT], f32, tag="pt", name="tp")[:, : 2 * E]
                    nc.tensor.transpose(
                        tp[:], lgs16[g][:, c * P : (c + 1) * P],
                        idsb[: 2 * E, : 2 * E],
                    )
                    nc.vector.tensor_copy(ltk16[:, c, :], tp[:])
                ltk16s.append(ltk16)

            # PE filler while the DVE top-2 chain runs
            dr_fill(acc_e2[0], 1, 0, range(KO2))

            gates = []
            for g in range(NGT):
                ltk16 = ltk16s[g]
                ltk = gp.tile([P, NGC, E], f32, tag="ltk", name=f"ltk{g}")
                nc.vector.tensor_tensor(
                    ltk[:], ltk16[:, :, :E], ltk16[:, :, E:], Alu.add
                )
                m1 = gp.tile([P, NGC, 1], f32, tag="m1")
                nc.vector.tensor_reduce(m1[:], ltk[:], mybir.AxisListType.X, Alu.max)
                mask1 = gp.tile([P, NGC, E], f32, tag="mask1")
                nc.vector.tensor_tensor(
                    mask1[:], ltk[:], m1.to_broadcast((P, NGC, E)), Alu.is_equal
                )
                l2 = gp.tile([P, NGC, E], f32, tag="l2")
                nc.vector.scalar_tensor_tensor(
                    l2[:], mask1[:], -1e30, ltk[:], Alu.mult, Alu.add
                )
                m2 = gp.tile([P, NGC, 1], f32, tag="m2")
                nc.vector.tensor_reduce(m2[:], l2[:], mybir.AxisListType.X, Alu.max)
                mask2 = gp.tile([P, NGC, E], f32, tag="mask2")
                nc.vector.tensor_tensor(
                    mask2[:], l2[:], m2.to_broadcast((P, NGC, E)), Alu.is_equal
                )
                dlt = gp.tile([P, NGC, 1], f32, tag="dlt")
                nc.vector.tensor_tensor(dlt[:], m2[:], m1[:], Alu.subtract)
                dlts = gp.tile([P, NGC, 1], f32, tag="dlts")
                nc.vector.tensor_scalar(
                    dlts[:], dlt[:], 1.0 / 64.0, 0.0, Alu.mult, Alu.add
                )
                g2 = gp.tile([P, NGC, 1], f32, tag="g2")
                nc.scalar.activation(g2[:], dlts[:], Act.Sigmoid)
                g1 = gp.tile([P, NGC, 1], f32, tag="g1")
                nc.vector.tensor_scalar(g1[:], g2[:], -1.0, 1.0, Alu.mult, Alu.add)

                gate = gp.tile([P, NGC, E], f32, tag="gate", name=f"gate{g}")
                nc.vector.tensor_tensor(
                    gate[:], mask1[:], g1.to_broadcast((P, NGC, E)), Alu.mult
                )
                gm2 = gp.tile([P, NGC, E], f32, tag="gm2")
                nc.vector.tensor_tensor(
                    gm2[:], mask2[:], g2.to_broadcast((P, NGC, E)), Alu.mult
                )
                nc.vector.tensor_tensor(gate[:], gate[:], gm2[:], Alu.add)
                gates.append(gate)

            # second PE filler group
            dr_fill(acc_e2[1], 1, 1, range(KO2))

            for g in range(NGT):
                gs = slice(g * GT, (g + 1) * GT)
                # transpose gates back to expert-major [8, 512]
                gts = gp.tile([E, GT], f32, tag="gts", name=f"gts{g}")
                for c in range(NGC):
                    tp2 = ps_t.tile([P, GT], f32, tag="pt", name="tp2")[:E, :P]
                    nc.tensor.transpose(tp2[:], gates[g][:, c, :], idsb[:])
                    nc.vector.tensor_copy(gts[:, c * P : (c + 1) * P], tp2[:])

                # expand expert gates (x scaling*64, folded into e8) to
                # the 128 rank slots: RG = e8.T @ gts
                RG = ps_t.tile([P, GT], f32, tag="pt", name="RG")
                nc.tensor.matmul(
                    RG[:], lhsT=e8sb[:], rhs=gts[:], start=True, stop=True
                )
                nc.vector.tensor_copy(rgp[:, gs], RG[:])
                # gated rank activations for this token tile (g == tt)
                nc.vector.tensor_tensor(
                    hwsb[:, gs], h_t[g][:], rgp[:, gs], Alu.mult
                )

            # ---- close the four early groups: lora term + epilogue ----
            for ot, accs in ((0, acc_e), (1, acc_e2)):
                os_ = slice(ot * P, (ot + 1) * P)
                for tt in range(2):
                    ts = slice(tt * TTILE, (tt + 1) * TTILE)
                    nc.tensor.matmul(
                        accs[tt][:],
                        lhsT=lbsb[:, os_],
                        rhs=hwsb[:, ts],
                        start=False,
                        stop=True,
                    )
                    osb = ob.tile([P, TTILE], f16, tag="osb", name="osbe")
                    nc.vector.scalar_tensor_tensor(
                        osb[:],
                        accs[tt][:],
                        1.0 / W8_SCALE,
                        bbsb[:, ot, None].to_broadcast((P, TTILE)),
                        Alu.mult,
                        Alu.add,
                    )
                    nc.sync.dma_start(yt_v[:, ot, ts], osb[:])

            phase1.close()

            # ---- phase 2: remaining base matmul + fused lora_B ----
            phase2 = ExitStack()
            ps_o = phase2.enter_context(
                tc.tile_pool(name="ps_o", bufs=6, space="PSUM")
            )

            for ot in range(2, OTILES):
                # keep the weight stream ~3 tiles ahead, alternating queues
                pre = ot + 3
                if pre < OTILES and w_sb[pre] is None:
                    w_sb[pre] = wpool.tile(
                        [P, KO2, 2, P], f8, tag="w", name=f"w{pre}"
                    )
                    eng = nc.scalar if pre % 2 == 0 else nc.gpsimd
                    eng.dma_start(w_sb[pre][:], wt_v[:, pre, :, :, :])
                os_ = slice(ot * P, (ot + 1) * P)
                for tt in range(NT):
                    ts = slice(tt * TTILE, (tt + 1) * TTILE)
                    acc = ps_o.tile([P, TTILE], f32, tag="acc")
                    for k2 in range(KO2):
                        nc.tensor.matmul(
                            acc[:],
                            lhsT=w_sb[ot][:, k2, :, :],
                            rhs=x8sb[:, 2 * k2 : 2 * k2 + 2, ts],
                            start=(k2 == 0),
                            stop=False,
                            perf_mode=DR,
                        )
                    nc.tensor.matmul(
                        acc[:],
                        lhsT=lbsb[:, os_],
                        rhs=hwsb[:, ts],
                        start=False,
                        stop=True,
                    )
                    osb = ob.tile([P, TTILE], f16, tag="osb")
                    # acc holds 64x(base+lora); rescale + bias in one op
                    nc.vector.scalar_tensor_tensor(
                        osb[:],
                        acc[:],
                        1.0 / W8_SCALE,
                        bbsb[:, ot, None].to_broadcast((P, TTILE)),
                        Alu.mult,
                        Alu.add,
                    )
                    nc.sync.dma_start(yt_v[:, ot, ts], osb[:])
            phase2.close()

    nc.compile()
    return nc


def get_program():
    if "nc" not in _prog_cache:
        _prog_cache["nc"] = _build_program()
    return _prog_cache["nc"]


def make_in_maps(x, base_w, base_b, lora_A, lora_B, router_w, scalings):
    """Host-side sharding/layout prep -> per-core input dicts."""
    import ml_dtypes

    x = np.ascontiguousarray(x, dtype=np.float32)
    # partition-major layout [P, KO, T]: per-core 4-ko DMA blocks are
    # 1MB with 8KB contiguous runs per partition
    xt_full = np.ascontiguousarray(
        x.reshape(T, KO, P).transpose(2, 1, 0)
    )  # [P, KO, T]

    # base weights x64 -> e4m3, DoubleRow pair layout [ot,p,k2,j,f]
    wt_host = np.ascontiguousarray(
        (base_w.T.astype(np.float32) * W8_SCALE)
        .reshape(KO2, 2, P, OTILES, P)
        .transpose(3, 2, 0, 1, 4)
        .reshape(OTILES * P, KO2 * 2 * P)
        .astype(ml_dtypes.float8_e4m3)
    )
    lb_host = np.ascontiguousarray(
        lora_B.T.astype(np.float32).astype(ml_dtypes.bfloat16)
    )

    # lora_A.T (unscaled; scaling folded into e8) -> [p, ko*128+r]
    ar_host = np.ascontiguousarray(
        lora_A.T.astype(np.float32)
        .reshape(KO, P, R)
        .transpose(1, 0, 2)
        .reshape(P, KO * R)
        .astype(np.float16)
    )

    # router_w.T x64 -> [p, ko, 16]: cols 0:8 = fp16 hi, 8:16 = fp16 lo
    # (hi + lo == 64*rw to ~2^-24 relative)
    rw64 = np.ascontiguousarray(
        router_w.T.astype(np.float32)
        .reshape(KO, P, E)
        .transpose(1, 0, 2)
    ) * np.float32(64.0)                                  # [P, KO, E]
    rwh_host = rw64.astype(np.float16)
    rwr_host = (rw64 - rwh_host.astype(np.float32)).astype(np.float16)
    rw2_host = np.ascontiguousarray(
        np.concatenate([rwh_host, rwr_host], axis=-1).reshape(P, KO * 2 * E)
    )

    # expert -> rank-slot expansion with per-expert scaling and the x64
    # fp8 weight scale folded in (so the lora matmul accumulates at the
    # same scale as the fp8 base steps)
    e8 = np.zeros((E, P), dtype=np.float32)
    s = np.asarray(scalings, dtype=np.float32) * W8_SCALE
    for e in range(E):
        e8[e, e * RANK : (e + 1) * RANK] = s[e]
    idm = np.eye(P, dtype=np.float32)
    bbf = base_b.astype(np.float32)

    xh_full = xt_full.astype(np.float16)
    rh_full = (xt_full - xh_full.astype(np.float32)).astype(np.float16)

    in_maps = []
    for c in range(N_CORES):
        cs = slice(c * T_PC, (c + 1) * T_PC)
        m = {
            "xh": np.ascontiguousarray(xh_full[:, :, cs]).reshape(P, KO * T_PC),
            "rh": np.ascontiguousarray(rh_full[:, :, cs]).reshape(P, KO * T_PC),
            "wt": wt_host,
            "lb": lb_host,
            "ar": ar_host,
            "rw2": rw2_host,
            "bb": bbf,
            "e8": e8,
            "idm": idm,
        }
        in_maps.append(m)
    return in_maps


def assemble_output(results):
    """Per-core yt [O, T_PC] fp16 -> full [B, S, O] f32."""
    yt_full = np.concatenate(
        [np.asarray(r["yt"]) for r in results], axis=1
    )  # [O, T] fp16
    return np.ascontiguousarray(yt_full.T.astype(np.float32)).reshape(B, S, O)


def kernel(**inputs):
    _ensure_path()
    from concourse.bass_utils import run_bass_kernel_spmd

    assert int(inputs["top_k"]) == 2
    nc = get_program()
    in_maps = make_in_maps(
        inputs["x"],
        inputs["base_w"],
        inputs["base_b"],
        inputs["lora_A"],
        inputs["lora_B"],
        inputs["router_w"],
        inputs["scalings"],
    )
    res = run_bass_kernel_spmd(nc, in_maps, list(range(N_CORES)))
    return assemble_output(res.results)


if __name__ == "__main__":
    # quick smoke: build the program only
    get_program()
    print("program built OK")
